# revision 1
# baseline (speedup 1.0000x reference)
"""Trainium2 Bass kernel for nn_AttentionHead (B=32, C=256, H=W=32).

Reference computation (per batch b):
    xs = x[b].reshape(C, S).T                     # [S, C], S = H*W = 1024
    q = xs @ wq.T + bq ; k = xs @ wk.T + bk ; v = xs @ wv.T + bv
    attn = softmax(q @ k.T / sqrt(C), axis=-1)    # [S, S]
    out[b] = silu(attn @ v).T.reshape(C, H, W)

Sharding: data-parallel over B across 8 cores (4 batches/core); the three
CxC projection weights are replicated.

Device-side layout choices (all matmuls in fp32r, full-rate fp32):
  - x[b] stays in its native [C, S] layout; it directly serves as the
    matmul rhs for qT/kT ([d, s] layouts) and as lhsT for v ([t, d]).
  - scores are computed transposed, scoresT[t, s], so softmax's exp is a
    plain elementwise ACT op and the softmax *sum* over t is obtained via
    an extra column appended to v (v_ext[:, 256] == 2): the attn@v matmul
    then yields 2*denominator as column 256 of its own output.  No
    max-subtraction is needed: |logits| <= ~8 here, exp is safe in fp32.
  - v's bias (and the denominator column's 2.0) is added during PSUM
    eviction against a partition-broadcast bias tile.
  - the final tile is out[s, d] = silu(z), z = pv/denom, computed via
    tanh so Exp and Tanh share one ACT function-table set (SiLU's set
    would force a ~1.3-2.7us table reload per switch):
        h = pv * rec         (rec = 1/(2*denom), per-partition scalar)
        out = h * tanh(h) + h
  - stage 2 (scores+exp) runs n-chunk-major and stage 3 (attn@v) is
    interleaved per n-chunk so the PE never waits long for E evictions.
The [B, S, C] device output is transposed to [B, C, H, W] on the host.
"""

import numpy as np

import concourse.tile as tile
from concourse import bacc, mybir
from concourse.bass_utils import run_bass_kernel_spmd

F32 = mybir.dt.float32
F32R = mybir.dt.float32r
AF = mybir.ActivationFunctionType

B, C, H, W = 32, 256, 32, 32
S = H * W              # 1024
N_CORES = 8
BPC = B // N_CORES     # 4 batches per core
CT = C // 128          # 2 contraction tiles
DT = C // 128          # 2 output-channel tiles
TT = S // 128          # 8 key/query row tiles
NS = S // 512          # 2 512-wide column chunks
SPH = TT // NS         # 4 s-tiles per n-chunk
SCALE = 1.0 / 16.0     # 1/sqrt(C)


def _build_attention_core(iters=1, loop_n=None):
    nc = bacc.Bacc("TRN2", debug=False)

    x_d = nc.dram_tensor("x", [BPC, C, S], F32R, kind="ExternalInput")
    wq_d = nc.dram_tensor("wq_t", [C, C], F32R, kind="ExternalInput")
    wk_d = nc.dram_tensor("wk_t", [C, C], F32R, kind="ExternalInput")
    wv_d = nc.dram_tensor("wv_e", [C, C + 2], F32R, kind="ExternalInput")
    bv_d = nc.dram_tensor("bv_e", [1, C + 2], F32R, kind="ExternalInput")
    bq_d = nc.dram_tensor("bq_p", [128, DT], F32, kind="ExternalInput")
    bk_d = nc.dram_tensor("bk_p", [128, DT], F32, kind="ExternalInput")
    out_d = nc.dram_tensor("out", [BPC, S, C], F32, kind="ExternalOutput")

    with tile.TileContext(nc) as tc:
        with (
            tc.tile_pool(name="consts", bufs=1) as consts,
            tc.tile_pool(name="xp", bufs=2) as xp,
            tc.tile_pool(name="qp", bufs=2) as qp,
            tc.tile_pool(name="kp", bufs=2) as kp,
            tc.tile_pool(name="vp", bufs=2) as vp,
            tc.tile_pool(name="ep", bufs=2) as ep,
            tc.tile_pool(name="op", bufs=4) as op,
            tc.tile_pool(name="ps_vo", bufs=2, space="PSUM") as ps_vo,
            tc.tile_pool(name="ps_s", bufs=3, space="PSUM") as ps_s,
        ):
            def load_x(b):
                x_sb = xp.tile([128, CT, S], F32R, name=f"x_{b}", tag="x")
                for ct in range(CT):
                    for n in range(NS):
                        nc.sync.dma_start(
                            out=x_sb[:, ct, n * 512:(n + 1) * 512],
                            in_=x_d.ap()[b, ct * 128:(ct + 1) * 128,
                                         n * 512:(n + 1) * 512],
                        )
                return x_sb

            # DMA emission order tracks the first matmuls' needs: wq and the
            # two n=0 chunks of batch-0 x unblock the first projection group.
            wq_sb = consts.tile([128, CT, C], F32R)
            wk_sb = consts.tile([128, CT, C], F32R)
            wv_sb = consts.tile([128, CT, C + 2], F32R)
            bv_sb = consts.tile([128, C + 2], F32R)
            bq_sb = consts.tile([128, DT], F32)
            bk_sb = consts.tile([128, DT], F32)
            x_pref = xp.tile([128, CT, S], F32R, name="x_pref", tag="x")
            # warm the PE HAM clock while the first DMAs are in flight:
            # tiny matmuls on a zeroed scratch tile, results unused
            warm_sb = consts.tile([1, 128], F32)
            nc.vector.memset(warm_sb, 0.0)
            warm_ps = ps_s.tile([128, NS, 512], F32, name="warm_ps", tag="pss")
            for _ in range(8):
                nc.tensor.matmul(
                    warm_ps[0:1, 0, 0:128], warm_sb[0:1, 0:1], warm_sb[0:1, :],
                    start=True, stop=True)
            nc.sync.dma_start(out=wq_sb, in_=wq_d.ap().rearrange("(ct p) d -> p ct d", p=128))
            for ct in range(CT):
                nc.sync.dma_start(
                    out=x_pref[:, ct, 0:512], in_=x_d.ap()[0, ct * 128:(ct + 1) * 128, 0:512],
                )
            nc.sync.dma_start(out=wv_sb, in_=wv_d.ap().rearrange("(ct p) d -> p ct d", p=128))
            for ct in range(CT):
                nc.sync.dma_start(
                    out=x_pref[:, ct, 512:1024], in_=x_d.ap()[0, ct * 128:(ct + 1) * 128, 512:1024],
                )
            nc.sync.dma_start(out=wk_sb, in_=wk_d.ap().rearrange("(ct p) d -> p ct d", p=128))
            nc.sync.dma_start(out=bq_sb, in_=bq_d.ap())
            nc.sync.dma_start(out=bk_sb, in_=bk_d.ap())
            nc.sync.dma_start(out=bv_sb, in_=bv_d.ap().to_broadcast([128, C + 2]))

            import contextlib
            loop_cm = tc.For_i(0, loop_n, 1) if loop_n else contextlib.nullcontext()
            with loop_cm:
              for it in range(BPC * iters):
                b = it % BPC
                x_sb = x_pref if (it == 0 and not loop_n) else load_x(b)

                # ---- qT[d, s], kT[d, s] = w.T-slices @ x (+ per-part. bias) ----
                q_sb = qp.tile([128, DT, S], F32R, name=f"q_{b}", tag="q")
                k_sb = kp.tile([128, DT, S], F32R, name=f"k_{b}", tag="k")
                for wi, (w_sb, b_sb, dst) in enumerate(
                        ((wq_sb, bq_sb, q_sb), (wk_sb, bk_sb, k_sb))):
                    for dt in range(DT):
                        pqk = ps_s.tile([128, NS, 512], F32, name="pqk", tag="pss")
                        for n in range(NS):
                            for ct in range(CT):
                                nc.tensor.matmul(
                                    pqk[:, n, :],
                                    w_sb[:, ct, dt * 128:(dt + 1) * 128],
                                    x_sb[:, ct, n * 512:(n + 1) * 512],
                                    start=(ct == 0),
                                    stop=(ct == CT - 1),
                                )
                        for n in range(NS):
                            if wi == 0 and dt == 0:
                                nc.scalar.activation(
                                    dst[:, dt, n * 512:(n + 1) * 512],
                                    pqk[:, n, :],
                                    AF.Identity, bias=b_sb[:, dt:dt + 1],
                                )
                            else:
                                nc.vector.tensor_scalar_add(
                                    dst[:, dt, n * 512:(n + 1) * 512],
                                    pqk[:, n, :],
                                    b_sb[:, dt:dt + 1],
                                )

                # ---- v_ext[t, d'] = x-slices.T @ wv_e, + bv (broadcast) ----
                v_sb = vp.tile([128, TT, C + 2], F32R, name=f"v_{b}", tag="v")
                for tt in range(TT):
                    pv = ps_vo.tile([128, C + 2], F32, name="pv", tag="vo")
                    for ct in range(CT):
                        nc.tensor.matmul(
                            pv,
                            x_sb[:, ct, tt * 128:(tt + 1) * 128],
                            wv_sb[:, ct, :],
                            start=(ct == 0),
                            stop=(ct == CT - 1),
                        )
                    nc.vector.tensor_tensor(
                        v_sb[:, tt, :], pv, bv_sb, op=mybir.AluOpType.add,
                    )

                # ---- per n-chunk: E[t, s-chunk] then out rows of that chunk ----
                e_sb = ep.tile([128, TT, S], F32R, name=f"e_{b}", tag="e")
                for tt in range(TT):
                    pss = ps_s.tile([128, NS, 512], F32, name="pss", tag="pss")
                    for n in range(NS):
                        for dt in range(DT):
                            nc.tensor.matmul(
                                pss[:, n, :],
                                k_sb[:, dt, tt * 128:(tt + 1) * 128],
                                q_sb[:, dt, n * 512:(n + 1) * 512],
                                start=(dt == 0),
                                stop=(dt == DT - 1),
                            )
                    nc.scalar.activation(
                        e_sb[:, tt, :], pss.rearrange("p n f -> p (n f)"), AF.Exp,
                        scale=SCALE,
                    )

                for n in range(NS):
                    for st in range(n * SPH, (n + 1) * SPH):
                        po = ps_vo.tile([128, C + 2], F32, name="po", tag="vo")
                        for tt in range(TT):
                            nc.tensor.matmul(
                                po,
                                e_sb[:, tt, st * 128:(st + 1) * 128],
                                v_sb[:, tt, :],
                                start=(tt == 0),
                                stop=(tt == TT - 1),
                            )
                        rec = op.tile([128, 1], F32, name="rec", tag="rec")
                        nc.vector.reciprocal(rec, po[:, C:C + 1])
                        hh = op.tile([128, C], F32, name="hh", tag="hh")
                        nc.vector.tensor_scalar_mul(hh, po[:, :C], rec)
                        th = op.tile([128, C], F32, name="th", tag="th")
                        nc.scalar.activation(th, hh, AF.Tanh)
                        last = it == BPC * iters - 1
                        eng = nc.vector if (last and st % 2 == 1) else nc.gpsimd
                        hth = op.tile([128, C], F32, name="hth", tag="hth")
                        eng.tensor_tensor(hth, hh, th, op=mybir.AluOpType.mult)
                        o_sb = op.tile([128, C], F32, name="o_sb", tag="o")
                        eng.tensor_tensor(o_sb, hth, hh, op=mybir.AluOpType.add)
                        nc.sync.dma_start(
                            out=out_d.ap()[b, st * 128:(st + 1) * 128, :], in_=o_sb,
                        )

    nc.compile()
    return nc


_NC_CACHE = None


def _get_nc():
    global _NC_CACHE
    if _NC_CACHE is None:
        _NC_CACHE = _build_attention_core()
    return _NC_CACHE


def _make_in_maps(x, wq, bq, wk, bk, wv, bv):
    x = np.ascontiguousarray(x, dtype=np.float32).reshape(B, C, S)
    wq_t = np.ascontiguousarray(wq.T, dtype=np.float32)
    wk_t = np.ascontiguousarray(wk.T, dtype=np.float32)
    wv_e = np.zeros((C, C + 2), dtype=np.float32)
    wv_e[:, :C] = wv.T
    bv_e = np.zeros((1, C + 2), dtype=np.float32)
    bv_e[0, C] = 2.0
    bv_e[0, :C] = bv
    bq_p = np.ascontiguousarray(bq.reshape(DT, 128).T)
    bk_p = np.ascontiguousarray(bk.reshape(DT, 128).T)
    shared = {
        "wq_t": wq_t, "wk_t": wk_t, "wv_e": wv_e, "bv_e": bv_e,
        "bq_p": bq_p, "bk_p": bk_p,
    }
    return [
        {"x": x[i * BPC:(i + 1) * BPC], **shared} for i in range(N_CORES)
    ]


def kernel(x, wq, bq, wk, bk, wv, bv, _trace=False):
    nc = _get_nc()
    in_maps = _make_in_maps(
        np.asarray(x), np.asarray(wq), np.asarray(bq), np.asarray(wk),
        np.asarray(bk), np.asarray(wv), np.asarray(bv),
    )
    try:
        res = run_bass_kernel_spmd(nc, in_maps, list(range(N_CORES)), trace=_trace)
    except Exception:
        # the axon-tunneled devices occasionally report a transient
        # NRT_EXEC_UNIT_UNRECOVERABLE right after another process exited;
        # one delayed retry has always recovered in practice
        import time
        time.sleep(15)
        res = run_bass_kernel_spmd(nc, in_maps, list(range(N_CORES)), trace=_trace)
    out = np.concatenate([res.results[i]["out"] for i in range(N_CORES)], axis=0)
    out = out.transpose(0, 2, 1).reshape(B, C, H, W)
    if _trace:
        return np.ascontiguousarray(out, dtype=np.float32), res
    return np.ascontiguousarray(out, dtype=np.float32)



# revision 5
# speedup vs baseline: 1.4902x; 1.4902x over previous
"""Trainium2 Bass kernel for nn_AttentionHead (B=32, C=256, H=W=32).

Reference computation (per batch b):
    xs = x[b].reshape(C, S).T                     # [S, C], S = H*W = 1024
    q = xs @ wq.T + bq ; k = xs @ wk.T + bk ; v = xs @ wv.T + bv
    attn = softmax(q @ k.T / sqrt(C), axis=-1)    # [S, S]
    out[b] = silu(attn @ v).T.reshape(C, H, W)

Sharding: data-parallel over B across 8 cores (4 batches/core); the small
projection weights are replicated.

Algebraic restructure (exact up to softmax's shift invariance):
    q @ k.T  =  xs @ (wq.T @ wk) @ xs.T  +  1*(xs @ wk.T @ bq).T  +  f(s)*1.T
  softmax over keys t drops the per-query constants f(s), so the device only
  computes G = xs @ Mm (Mm = wq.T @ wk, host-precomputed) and the per-key
  bias kb = xs @ c (c = wk.T @ bq / sqrt(C), host-precomputed) -- one
  projection matmul instead of two (-9% PE work).

Device-side layout (all matmuls bf16 inputs, fp32 PSUM accumulate):
  - x[b] stays in native [C, S] layout; it serves as the matmul rhs for
    GT ([d, s]) and as lhsT for both v ([t, d']) and scoresT ([t, s]).
  - v_ext[t, 0:C] = v; col C == 2 (denominator column: attn@v_ext yields
    2*sum_t e as col C); col C+1 = kb[t]/sqrt(C) (per-key softmax bias,
    consumed as the Exp activation's per-partition bias operand).
  - scoresT[t, s] needs no max-subtraction: |logits| <= ~9, exp safe in
    fp32; e is stored bf16 (max ~e^9, fine).
  - final tile out[s, d] = silu(z), z = pv/denom, via tanh so Exp and Tanh
    share one ACT function-table set:  h = pv * rec  (rec = 1/(2*denom)),
    out = h*tanh(h) + h.  Output stored bf16, upconverted on host.
  - PSUM evictions are spread across ACT/DVE/Pool so no single engine gates
    the PE; x for batch b+1 is DMA-prefetched during batch b's scores.
The [B, S, C] device output is transposed to [B, C, H, W] on the host.
"""

import numpy as np
import ml_dtypes

import concourse.tile as tile
from concourse import bacc, mybir
from concourse.bass_utils import run_bass_kernel_spmd

F32 = mybir.dt.float32
BF16 = mybir.dt.bfloat16
AF = mybir.ActivationFunctionType

B, C, H, W = 32, 256, 32, 32
S = H * W              # 1024
N_CORES = 8
BPC = B // N_CORES     # 4 batches per core
CT = C // 128          # 2 contraction tiles
DT = C // 128          # 2 output-channel tiles
TT = S // 128          # 8 key/query row tiles
NS = S // 512          # 2 512-wide column chunks
CE = C + 2             # v_ext columns: v | 2.0-denominator | kb
SCALE = 1.0 / 16.0     # 1/sqrt(C)


def _build_attention_core(iters=1, loop_n=None):
    nc = bacc.Bacc("TRN2", debug=False)

    x_d = nc.dram_tensor("x", [BPC, C, S], BF16, kind="ExternalInput")
    m_d = nc.dram_tensor("m_t", [C, C], BF16, kind="ExternalInput")
    wv_d = nc.dram_tensor("wv_e", [C, CE], BF16, kind="ExternalInput")
    bv_d = nc.dram_tensor("bv_e", [1, CE], F32, kind="ExternalInput")
    out_d = nc.dram_tensor("out", [BPC, S, C], BF16, kind="ExternalOutput")

    with tile.TileContext(nc) as tc:
        with (
            tc.tile_pool(name="consts", bufs=1) as consts,
            tc.tile_pool(name="xp", bufs=2) as xp,
            tc.tile_pool(name="gp", bufs=2) as gp,
            tc.tile_pool(name="vp", bufs=2) as vp,
            tc.tile_pool(name="ep", bufs=2) as ep,
            tc.tile_pool(name="op", bufs=4) as op,
            tc.tile_pool(name="ob", bufs=2) as ob,
            tc.tile_pool(name="ps_vo", bufs=2, space="PSUM") as ps_vo,
            tc.tile_pool(name="ps_s", bufs=3, space="PSUM") as ps_s,
        ):
            def load_x(b, split=False):
                x_sb = xp.tile([128, CT, S], BF16, name=f"x_{b}", tag="x")
                if split:
                    # chunks in first-consumption order (stage A: n-major)
                    for n in range(NS):
                        for ct in range(CT):
                            nc.sync.dma_start(
                                out=x_sb[:, ct, n * 512:(n + 1) * 512],
                                in_=x_d.ap()[b, ct * 128:(ct + 1) * 128,
                                             n * 512:(n + 1) * 512],
                            )
                else:
                    nc.sync.dma_start(
                        out=x_sb,
                        in_=x_d.ap()[b].rearrange("(ct p) s -> p ct s", p=128),
                    )
                return x_sb

            m_sb = consts.tile([128, CT, C], BF16)
            wv_sb = consts.tile([128, CT, CE], BF16)
            bv_sb = consts.tile([128, CE], F32)
            x_pref = xp.tile([128, CT, S], BF16, name="x_pref", tag="x")
            # warm the PE HAM clock while the first DMAs are in flight
            warm_sb = consts.tile([1, 128], F32)
            nc.vector.memset(warm_sb, 0.0)
            warm_ps = ps_s.tile([128, NS, 512], F32, name="warm_ps", tag="pss")
            for _ in range(8):
                nc.tensor.matmul(
                    warm_ps[0:1, 0, 0:128], warm_sb[0:1, 0:1], warm_sb[0:1, :],
                    start=True, stop=True)
            nc.sync.dma_start(out=m_sb, in_=m_d.ap().rearrange("(ct p) d -> p ct d", p=128))
            for ct in range(CT):
                nc.sync.dma_start(
                    out=x_pref[:, ct, 0:512], in_=x_d.ap()[0, ct * 128:(ct + 1) * 128, 0:512],
                )
            nc.sync.dma_start(out=wv_sb, in_=wv_d.ap().rearrange("(ct p) d -> p ct d", p=128))
            for ct in range(CT):
                nc.sync.dma_start(
                    out=x_pref[:, ct, 512:1024], in_=x_d.ap()[0, ct * 128:(ct + 1) * 128, 512:1024],
                )
            nc.sync.dma_start(out=bv_sb, in_=bv_d.ap().to_broadcast([128, CE]))

            import contextlib
            loop_cm = tc.For_i(0, loop_n, 1) if loop_n else contextlib.nullcontext()
            with loop_cm:
              for it in range(BPC * iters):
                b = it % BPC
                if it == 0:
                    x_sb = x_pref if not loop_n else load_x(0, split=True)

                # ---- stage A: GT[d, s] = Mm.T-slices @ x ----
                g_sb = gp.tile([128, DT, S], BF16, name=f"g_{b}", tag="g")
                ev_engines = (nc.scalar, nc.vector, nc.scalar, nc.vector)
                for dt in range(DT):
                    pqk = ps_s.tile([128, NS, 512], F32, name="pqk", tag="pss")
                    for n in range(NS):
                        for ct in range(CT):
                            nc.tensor.matmul(
                                pqk[:, n, :],
                                m_sb[:, ct, dt * 128:(dt + 1) * 128],
                                x_sb[:, ct, n * 512:(n + 1) * 512],
                                start=(ct == 0),
                                stop=(ct == CT - 1),
                            )
                    for n in range(NS):
                        eng = ev_engines[dt * NS + n]
                        if eng is nc.scalar:
                            nc.scalar.activation(
                                g_sb[:, dt, n * 512:(n + 1) * 512],
                                pqk[:, n, :], AF.Identity,
                            )
                        else:
                            eng.tensor_copy(
                                g_sb[:, dt, n * 512:(n + 1) * 512],
                                pqk[:, n, :],
                            )

                # ---- stage BC (interleaved per tt): v_ext tile, then scoresT
                # tile + exp.  The small v matmul group (258 rows) slots
                # between score groups so the DVE bias-add eviction of v(tt)
                # gets a full score-group window before pv's bank is reused.
                #   v_ext[t, d'] = x-slices.T @ wv_e, + bv_e (DVE)
                #   e[t, s] = exp(scoresT/16 + kb/16)       (ACT)
                v_sb = vp.tile([128, TT, CE], BF16, name=f"v_{b}", tag="v")
                e_sb = ep.tile([128, TT, S], BF16, name=f"e_{b}", tag="e")
                for tt in range(TT):
                    pv = ps_vo.tile([128, CE], F32, name="pv", tag="vo")
                    for ct in range(CT):
                        nc.tensor.matmul(
                            pv,
                            x_sb[:, ct, tt * 128:(tt + 1) * 128],
                            wv_sb[:, ct, :],
                            start=(ct == 0),
                            stop=(ct == CT - 1),
                        )
                    pss = ps_s.tile([128, NS, 512], F32, name="pss", tag="pss")
                    for n in range(NS):
                        for ct in range(CT):
                            nc.tensor.matmul(
                                pss[:, n, :],
                                x_sb[:, ct, tt * 128:(tt + 1) * 128],
                                g_sb[:, ct, n * 512:(n + 1) * 512],
                                start=(ct == 0),
                                stop=(ct == CT - 1),
                            )
                    nc.vector.tensor_tensor(
                        v_sb[:, tt, :], pv, bv_sb, op=mybir.AluOpType.add,
                    )
                    nc.scalar.activation(
                        e_sb[:, tt, :], pss.rearrange("p n f -> p (n f)"), AF.Exp,
                        scale=SCALE, bias=v_sb[:, tt, C + 1:C + 2],
                    )
                    if tt == 0 and it + 1 < BPC * iters:
                        x_next = load_x((it + 1) % BPC)

                # ---- stage D: out rows; out = h*tanh(h) + h, h = pv/(2 den) ----
                o_sb = ob.tile([128, TT, C], BF16, name=f"o_{b}", tag="o")
                for st in range(TT):
                    po = ps_vo.tile([128, CE], F32, name="po", tag="vo")
                    for tt in range(TT):
                        nc.tensor.matmul(
                            po,
                            e_sb[:, tt, st * 128:(st + 1) * 128],
                            v_sb[:, tt, :],
                            start=(tt == 0),
                            stop=(tt == TT - 1),
                        )
                    rec = op.tile([128, 1], F32, name="rec", tag="rec")
                    nc.vector.reciprocal(rec, po[:, C:C + 1])
                    hh = op.tile([128, C], F32, name="hh", tag="hh")
                    nc.vector.tensor_scalar_mul(hh, po[:, :C], rec)
                    th = op.tile([128, C], F32, name="th", tag="th")
                    nc.scalar.activation(th, hh, AF.Tanh)
                    hth = op.tile([128, C], F32, name="hth", tag="hth")
                    nc.gpsimd.tensor_tensor(hth, hh, th, op=mybir.AluOpType.mult)
                    nc.gpsimd.tensor_tensor(
                        o_sb[:, st, :], hth, hh, op=mybir.AluOpType.add,
                    )
                nc.sync.dma_start(
                    out=out_d.ap()[b].rearrange("(st p) d -> p st d", p=128),
                    in_=o_sb,
                )
                if it + 1 < BPC * iters:
                    x_sb = x_next

    nc.compile()
    return nc


_NC_CACHE = None


def _get_nc():
    global _NC_CACHE
    if _NC_CACHE is None:
        _NC_CACHE = _build_attention_core()
    return _NC_CACHE


def _make_in_maps(x, wq, bq, wk, bk, wv, bv):
    x = np.ascontiguousarray(x, dtype=np.float32).reshape(B, C, S)
    x_bf = x.astype(ml_dtypes.bfloat16)
    m_t = (wq.T.astype(np.float32) @ wk.astype(np.float32)).astype(ml_dtypes.bfloat16)
    c_vec = (wk.T.astype(np.float32) @ np.asarray(bq, dtype=np.float32)) * SCALE
    wv_e = np.zeros((C, CE), dtype=np.float32)
    wv_e[:, :C] = wv.T
    wv_e[:, C + 1] = c_vec
    wv_e = wv_e.astype(ml_dtypes.bfloat16)
    bv_e = np.zeros((1, CE), dtype=np.float32)
    bv_e[0, :C] = bv
    bv_e[0, C] = 2.0
    shared = {"m_t": m_t, "wv_e": wv_e, "bv_e": bv_e}
    return [
        {"x": x_bf[i * BPC:(i + 1) * BPC], **shared} for i in range(N_CORES)
    ]


def kernel(x, wq, bq, wk, bk, wv, bv, _trace=False):
    nc = _get_nc()
    in_maps = _make_in_maps(
        np.asarray(x), np.asarray(wq), np.asarray(bq), np.asarray(wk),
        np.asarray(bk), np.asarray(wv), np.asarray(bv),
    )
    try:
        res = run_bass_kernel_spmd(nc, in_maps, list(range(N_CORES)), trace=_trace)
    except Exception:
        # the axon-tunneled devices occasionally report a transient
        # NRT_EXEC_UNIT_UNRECOVERABLE right after another process exited;
        # one delayed retry has always recovered in practice
        import time
        time.sleep(15)
        res = run_bass_kernel_spmd(nc, in_maps, list(range(N_CORES)), trace=_trace)
    out = np.concatenate(
        [np.asarray(res.results[i]["out"]) for i in range(N_CORES)], axis=0
    ).astype(np.float32)
    out = out.transpose(0, 2, 1).reshape(B, C, H, W)
    if _trace:
        return np.ascontiguousarray(out, dtype=np.float32), res
    return np.ascontiguousarray(out, dtype=np.float32)


# revision 9
# speedup vs baseline: 2.2674x; 1.5215x over previous
"""Trainium2 Bass kernel for nn_AttentionHead (B=32, C=256, H=W=32).

Reference computation (per batch b):
    xs = x[b].reshape(C, S).T                     # [S, C], S = H*W = 1024
    q = xs @ wq.T + bq ; k = xs @ wk.T + bk ; v = xs @ wv.T + bv
    attn = softmax(q @ k.T / sqrt(C), axis=-1)    # [S, S]
    out[b] = silu(attn @ v).T.reshape(C, H, W)

Sharding: data-parallel over B across 8 cores (4 batches/core); the small
projection weights are replicated.

Algebraic restructure (exact up to softmax's shift invariance):
    q @ k.T  =  xs @ (wq.T @ wk) @ xs.T  +  1*(xs @ wk.T @ bq).T  +  f(s)*1.T
  softmax over keys t drops the per-query constants f(s), so the device only
  computes G = xs @ Mm (Mm = wq.T @ wk, host-precomputed) and the per-key
  bias kb = xs @ c (c = wk.T @ bq / sqrt(C), host-precomputed) -- one
  projection matmul instead of two (-9% PE work).

Device-side layout (all matmuls bf16 inputs, fp32 PSUM accumulate):
  - x[b] stays in native [C, S] layout; it serves as the matmul rhs for
    GT ([d, s]) and as lhsT for both v ([t, d']) and scoresT ([t, s]).
  - v_ext[t, 0:C] = v; col C == 2 (denominator column: attn@v_ext yields
    2*sum_t e as col C); col C+1 = kb[t]/sqrt(C) (per-key softmax bias,
    consumed as the Exp activation's per-partition bias operand).
  - scoresT[t, s] needs no max-subtraction: |logits| <= ~9, exp safe in
    fp32; e is stored bf16 (max ~e^9, fine).
  - final tile out[s, d] = silu(z), z = pv/denom, via tanh so Exp and Tanh
    share one ACT function-table set:  h = pv * rec  (rec = 1/(2*denom)),
    out = h*tanh(h) + h.  Output stored bf16, upconverted on host.
  - PSUM evictions are spread across ACT/DVE/Pool so no single engine gates
    the PE; x for batch b+1 is DMA-prefetched during batch b's scores.
The [B, S, C] device output is transposed to [B, C, H, W] on the host.
"""

import numpy as np
import ml_dtypes

import concourse.tile as tile
from concourse import bacc, mybir
from concourse.bass_utils import run_bass_kernel_spmd

F32 = mybir.dt.float32
BF16 = mybir.dt.bfloat16
AF = mybir.ActivationFunctionType

B, C, H, W = 32, 256, 32, 32
S = H * W              # 1024
N_CORES = 8
BPC = B // N_CORES     # 4 batches per core
CT = C // 128          # 2 contraction tiles
DT = C // 128          # 2 output-channel tiles
TT = S // 128          # 8 key/query row tiles
NS = S // 512          # 2 512-wide column chunks
CE = C + 2             # v_ext columns: v | 2.0-denominator | kb
SCALE = 1.0 / 16.0     # 1/sqrt(C)


def _build_attention_core(iters=1, loop_n=None):
    nc = bacc.Bacc("TRN2", debug=False)

    x_d = nc.dram_tensor("x", [BPC, C, S], BF16, kind="ExternalInput")
    m_d = nc.dram_tensor("m_t", [C, C], BF16, kind="ExternalInput")
    wv_d = nc.dram_tensor("wv_e", [C, CE], BF16, kind="ExternalInput")
    bv_d = nc.dram_tensor("bv_e", [1, CE], F32, kind="ExternalInput")
    out_d = nc.dram_tensor("out", [BPC, S, C], BF16, kind="ExternalOutput")

    with tile.TileContext(nc) as tc:
        with (
            tc.tile_pool(name="consts", bufs=1) as consts,
            tc.tile_pool(name="xp", bufs=2) as xp,
            tc.tile_pool(name="gp", bufs=2) as gp,
            tc.tile_pool(name="vp", bufs=2) as vp,
            tc.tile_pool(name="ep", bufs=2) as ep,
            tc.tile_pool(name="op", bufs=4) as op,
            tc.tile_pool(name="ob", bufs=2) as ob,
            tc.tile_pool(name="ps_vo", bufs=2, space="PSUM") as ps_vo,
            tc.tile_pool(name="ps_s", bufs=3, space="PSUM") as ps_s,
        ):
            def load_x(b, split=False):
                x_sb = xp.tile([128, CT, S], BF16, name=f"x_{b}", tag="x")
                if split:
                    # chunks in first-consumption order (stage A: n-major)
                    for n in range(NS):
                        for ct in range(CT):
                            nc.sync.dma_start(
                                out=x_sb[:, ct, n * 512:(n + 1) * 512],
                                in_=x_d.ap()[b, ct * 128:(ct + 1) * 128,
                                             n * 512:(n + 1) * 512],
                            )
                else:
                    nc.sync.dma_start(
                        out=x_sb,
                        in_=x_d.ap()[b].rearrange("(ct p) s -> p ct s", p=128),
                    )
                return x_sb

            m_sb = consts.tile([128, CT, C], BF16)
            wv_sb = consts.tile([128, CT, CE], BF16)
            bv_sb = consts.tile([128, CE], F32)
            x_pref = xp.tile([128, CT, S], BF16, name="x_pref", tag="x")
            # warm the PE HAM clock while the first DMAs are in flight
            warm_sb = consts.tile([1, 128], F32)
            nc.vector.memset(warm_sb, 0.0)
            warm_ps = ps_s.tile([128, NS, 512], F32, name="warm_ps", tag="pss")
            for _ in range(8):
                nc.tensor.matmul(
                    warm_ps[0:1, 0, 0:128], warm_sb[0:1, 0:1], warm_sb[0:1, :],
                    start=True, stop=True)
            nc.sync.dma_start(out=m_sb, in_=m_d.ap().rearrange("(ct p) d -> p ct d", p=128))
            for ct in range(CT):
                nc.sync.dma_start(
                    out=x_pref[:, ct, 0:512], in_=x_d.ap()[0, ct * 128:(ct + 1) * 128, 0:512],
                )
            nc.sync.dma_start(out=wv_sb, in_=wv_d.ap().rearrange("(ct p) d -> p ct d", p=128))
            for ct in range(CT):
                nc.sync.dma_start(
                    out=x_pref[:, ct, 512:1024], in_=x_d.ap()[0, ct * 128:(ct + 1) * 128, 512:1024],
                )
            nc.sync.dma_start(out=bv_sb, in_=bv_d.ap().to_broadcast([128, CE]))

            import contextlib
            loop_cm = tc.For_i(0, loop_n, 1) if loop_n else contextlib.nullcontext()
            with loop_cm:
              for it in range(BPC * iters):
                b = it % BPC
                if it == 0:
                    x_sb = x_pref if not loop_n else load_x(0, split=True)

                # ---- stage A: GT[d, s] = Mm.T-slices @ x ----
                g_sb = gp.tile([128, DT, S], BF16, name=f"g_{b}", tag="g")
                # (dt, n) eviction engines chosen so both dt1 evictions run in
                # parallel the moment pqk(dt1) lands -- the first scores group
                # then never waits on a GT eviction
                ev_engines = (nc.vector, nc.scalar, nc.scalar, nc.vector)
                for dt in range(DT):
                    pqk = ps_s.tile([128, NS, 512], F32, name="pqk", tag="pss")
                    for n in range(NS):
                        for ct in range(CT):
                            nc.tensor.matmul(
                                pqk[:, n, :],
                                m_sb[:, ct, dt * 128:(dt + 1) * 128],
                                x_sb[:, ct, n * 512:(n + 1) * 512],
                                start=(ct == 0),
                                stop=(ct == CT - 1),
                            )
                    for n in range(NS):
                        eng = ev_engines[dt * NS + n]
                        if eng is nc.scalar:
                            nc.scalar.activation(
                                g_sb[:, dt, n * 512:(n + 1) * 512],
                                pqk[:, n, :], AF.Identity,
                            )
                        else:
                            eng.tensor_copy(
                                g_sb[:, dt, n * 512:(n + 1) * 512],
                                pqk[:, n, :],
                            )

                # ---- stage BC (interleaved per tt): v_ext tile, then scoresT
                # tile + exp.  The small v matmul group (258 rows) slots
                # between score groups so the DVE bias-add eviction of v(tt)
                # gets a full score-group window before pv's bank is reused.
                #   v_ext[t, d'] = x-slices.T @ wv_e, + bv_e (DVE)
                #   e[t, s] = exp(scoresT/16 + kb/16)       (ACT)
                v_sb = vp.tile([128, TT, CE], BF16, name=f"v_{b}", tag="v")
                e_sb = ep.tile([128, TT, S], BF16, name=f"e_{b}", tag="e")

                def v_tile(tt):
                    pv = ps_vo.tile([128, CE], F32, name="pv", tag="vo")
                    for ct in range(CT):
                        nc.tensor.matmul(
                            pv,
                            x_sb[:, ct, tt * 128:(tt + 1) * 128],
                            wv_sb[:, ct, :],
                            start=(ct == 0),
                            stop=(ct == CT - 1),
                        )
                    nc.vector.tensor_tensor(
                        v_sb[:, tt, :], pv, bv_sb, op=mybir.AluOpType.add,
                    )

                # two v tiles up front: they fill the PE while the GT
                # evictions land, so the first scores group never stalls
                v_tile(0)
                v_tile(1)
                for tt in range(TT):
                    pss = ps_s.tile([128, NS, 512], F32, name="pss", tag="pss")
                    for n in range(NS):
                        for ct in range(CT):
                            nc.tensor.matmul(
                                pss[:, n, :],
                                x_sb[:, ct, tt * 128:(tt + 1) * 128],
                                g_sb[:, ct, n * 512:(n + 1) * 512],
                                start=(ct == 0),
                                stop=(ct == CT - 1),
                            )
                    if tt + 2 < TT:
                        v_tile(tt + 2)
                    nc.scalar.activation(
                        e_sb[:, tt, :], pss.rearrange("p n f -> p (n f)"), AF.Exp,
                        scale=SCALE, bias=v_sb[:, tt, C + 1:C + 2],
                    )
                    if tt == 0 and it + 1 < BPC * iters:
                        x_next = load_x((it + 1) % BPC)

                # ---- stage D: out rows; out = h*tanh(h) + h, h = pv/(2 den) ----
                o_sb = ob.tile([128, TT, C], BF16, name=f"o_{b}", tag="o")
                for st in range(TT):
                    po = ps_vo.tile([128, CE], F32, name="po", tag="vo")
                    for tt in range(TT):
                        nc.tensor.matmul(
                            po,
                            e_sb[:, tt, st * 128:(st + 1) * 128],
                            v_sb[:, tt, :],
                            start=(tt == 0),
                            stop=(tt == TT - 1),
                        )
                    rec = op.tile([128, 1], F32, name="rec", tag="rec")
                    nc.vector.reciprocal(rec, po[:, C:C + 1])
                    hh = op.tile([128, C], F32, name="hh", tag="hh")
                    nc.vector.tensor_scalar_mul(hh, po[:, :C], rec)
                    th = op.tile([128, C], F32, name="th", tag="th")
                    nc.scalar.activation(th, hh, AF.Tanh)
                    hth = op.tile([128, C], F32, name="hth", tag="hth")
                    nc.gpsimd.tensor_tensor(hth, hh, th, op=mybir.AluOpType.mult)
                    nc.gpsimd.tensor_tensor(
                        o_sb[:, st, :], hth, hh, op=mybir.AluOpType.add,
                    )
                # stores ride GPSIMD's SWDGE so they never head-of-line block
                # the next body's x loads on the SP queue, and the issue cost
                # lands on the mostly-idle GPSIMD sequencer
                nc.gpsimd.dma_start(
                    out=out_d.ap()[b].rearrange("(st p) d -> p st d", p=128),
                    in_=o_sb,
                )
                if it + 1 < BPC * iters:
                    x_sb = x_next

    nc.compile()
    return nc


_NC_CACHE = None


def _get_nc():
    global _NC_CACHE
    if _NC_CACHE is None:
        _NC_CACHE = _build_attention_core()
    return _NC_CACHE


def _make_in_maps(x, wq, bq, wk, bk, wv, bv):
    x = np.ascontiguousarray(x, dtype=np.float32).reshape(B, C, S)
    x_bf = x.astype(ml_dtypes.bfloat16)
    m_t = (wq.T.astype(np.float32) @ wk.astype(np.float32)).astype(ml_dtypes.bfloat16)
    c_vec = (wk.T.astype(np.float32) @ np.asarray(bq, dtype=np.float32)) * SCALE
    wv_e = np.zeros((C, CE), dtype=np.float32)
    wv_e[:, :C] = wv.T
    wv_e[:, C + 1] = c_vec
    wv_e = wv_e.astype(ml_dtypes.bfloat16)
    bv_e = np.zeros((1, CE), dtype=np.float32)
    bv_e[0, :C] = bv
    bv_e[0, C] = 2.0
    shared = {"m_t": m_t, "wv_e": wv_e, "bv_e": bv_e}
    return [
        {"x": x_bf[i * BPC:(i + 1) * BPC], **shared} for i in range(N_CORES)
    ]


def kernel(x, wq, bq, wk, bk, wv, bv, _trace=False):
    nc = _get_nc()
    in_maps = _make_in_maps(
        np.asarray(x), np.asarray(wq), np.asarray(bq), np.asarray(wk),
        np.asarray(bk), np.asarray(wv), np.asarray(bv),
    )
    try:
        res = run_bass_kernel_spmd(nc, in_maps, list(range(N_CORES)), trace=_trace)
    except Exception:
        # the axon-tunneled devices occasionally report a transient
        # NRT_EXEC_UNIT_UNRECOVERABLE right after another process exited;
        # one delayed retry has always recovered in practice
        import time
        time.sleep(15)
        res = run_bass_kernel_spmd(nc, in_maps, list(range(N_CORES)), trace=_trace)
    out = np.concatenate(
        [np.asarray(res.results[i]["out"]) for i in range(N_CORES)], axis=0
    ).astype(np.float32)
    out = out.transpose(0, 2, 1).reshape(B, C, H, W)
    if _trace:
        return np.ascontiguousarray(out, dtype=np.float32), res
    return np.ascontiguousarray(out, dtype=np.float32)
